# revision 9
# baseline (speedup 1.0000x reference)
"""Domain-specific batchnorm (DSBatchNorm2 2D path) on 8 Trainium2 cores.

Strategy: feature-parallel sharding. Core c owns features [c*128,(c+1)*128).
Each core sees ALL cells for its features, so per-domain mean/var need no
cross-core reduction (counts come from the host). The host sorts cells by
domain and ships each core a transposed shard [128 features, N cells]; on
device the per-domain sums are contiguous free-axis reductions:

  pass1: per domain-run  ScalarE activation(Copy, accum_out)  -> sum(x)
                         VectorE tensor_tensor_reduce(x*x)    -> sum(x^2)
  stats: [128, 8] tiles  -> a = gamma*rsqrt(var+eps), b = beta - mean*a
  pass2: per domain-run  VectorE tensor_scalar(x*a + b)

In "fp16" mode x is shipped fp16 and stays resident in SBUF (one HBM read,
one write). In "fp32" mode everything is fp32 and pass2 re-reads from HBM.
"""

import os
from contextlib import ExitStack

import numpy as np

import concourse.bass as bass
import concourse.tile as tile
from concourse import bacc, mybir
from concourse.bass_utils import run_bass_kernel_spmd

N_DOMAIN = 8
EPS = 1e-5
NCORES = 8
P = 128  # SBUF partitions = features per core
CHUNK = 4096  # DMA/staging chunk (columns)
ALIGN = 8  # domain block alignment (columns)
TOTAL_ALIGN = 512
U_SPLIT = 0.85
GROUP = 4  # domains per finalize batch  # fraction of sum(x) columns reduced on VectorE (rest ScalarE)

MODE = os.environ.get("DSBN_MODE", "fp16")  # "fp16" | "fp32"

_cache: dict = {}


class _Plan:
    pass


def _plan(y: np.ndarray) -> _Plan:
    p = _Plan()
    y = np.asarray(y).astype(np.int64).ravel()
    n = y.shape[0]
    p.n = n
    p.counts = np.bincount(y, minlength=N_DOMAIN).astype(np.int64)
    p.order = np.argsort(y, kind="stable")
    blk = np.maximum((p.counts + ALIGN - 1) // ALIGN * ALIGN, ALIGN)
    np1 = int(blk.sum())
    npad = (np1 + TOTAL_ALIGN - 1) // TOTAL_ALIGN * TOTAL_ALIGN
    blk[-1] += npad - np1  # fold tail pad into the last domain's block
    p.npad = npad
    bstart = np.concatenate([[0], np.cumsum(blk)])[:-1]
    cstart = np.concatenate([[0], np.cumsum(p.counts)])[:-1]
    # column (padded position) of each domain-sorted row
    col_idx = np.empty(n, dtype=np.int64)
    for d in range(N_DOMAIN):
        col_idx[cstart[d] : cstart[d] + p.counts[d]] = bstart[d] + np.arange(
            p.counts[d]
        )
    p.col_idx = col_idx
    # chunks
    chunks = []
    cs = 0
    while cs < npad:
        cl = min(CHUNK, npad - cs)
        chunks.append((cs, cl))
        cs += cl
    p.chunks = chunks
    # runs = intersections of domain blocks with chunks, in column order
    runs = []  # (col_start, col_len, domain, chunk_index)
    dom_runs = [[] for _ in range(N_DOMAIN)]
    for ci, (cs, cl) in enumerate(chunks):
        ce = cs + cl
        for d in range(N_DOMAIN):
            rs = max(cs, int(bstart[d]))
            re_ = min(ce, int(bstart[d] + blk[d]))
            if rs < re_:
                dom_runs[d].append(len(runs))
                runs.append((rs, re_ - rs, d, ci))
    for d in range(N_DOMAIN):
        rr = dom_runs[d]
        assert rr == list(range(rr[0], rr[-1] + 1))
    p.runs = runs
    p.dom_runs = [(rr[0], rr[-1] + 1) for rr in dom_runs]
    return p


def _build(plan: _Plan, mode: str):
    fdt = mybir.dt.float16 if mode == "fp16" else mybir.dt.float32
    f32 = mybir.dt.float32
    A = mybir.AluOpType
    AF = mybir.ActivationFunctionType
    X = mybir.AxisListType.X
    resident = mode == "fp16"
    npad = plan.npad
    D = N_DOMAIN

    # per-chunk run lists and per-domain bookkeeping
    nch = len(plan.chunks)
    chunk_runs = [[] for _ in range(nch)]
    dom_nruns = [0] * D
    run_slot = []  # index of this run within its domain
    for rs, rl, d, ci in plan.runs:
        chunk_runs[ci].append((rs, rl, d))
        run_slot.append(dom_nruns[d])
        dom_nruns[d] += 1
    dom_last_chunk = [max(ci for rs, rl, dd, ci in plan.runs if dd == d) for d in range(D)]

    nc = bacc.Bacc("TRN2", target_bir_lowering=False, debug=False, num_devices=NCORES)
    xt = nc.dram_tensor("xt", [P, npad], fdt, kind="ExternalInput").ap()
    cmat = nc.dram_tensor("cmat", [P, 35], f32, kind="ExternalInput").ap()
    outd = nc.dram_tensor("out", [P, npad], fdt, kind="ExternalOutput").ap()

    with tile.TileContext(nc) as tc:
        with ExitStack() as ctx:
            const_p = ctx.enter_context(tc.tile_pool(name="const", bufs=1))
            xin_p = ctx.enter_context(
                tc.tile_pool(name="xin", bufs=1 if resident else 3)
            )
            scr_p = ctx.enter_context(tc.tile_pool(name="scr", bufs=2))
            st_p = ctx.enter_context(tc.tile_pool(name="st", bufs=1))
            fin_p = ctx.enter_context(tc.tile_pool(name="fin", bufs=1))
            out_p = ctx.enter_context(tc.tile_pool(name="ot", bufs=3))

            cm = const_p.tile([P, 35], f32, tag="cm")
            nc.sync.dma_start(cm[:], cmat)
            gam_col = cm[:, 32:33]
            bet_col = cm[:, 33:34]
            eps_col = cm[:, 34:35]

            # dummy Sqrt up front: pulls the ACT table load into the DMA ramp
            warm = const_p.tile([P, 1], f32, tag="warm")
            nc.scalar.activation(warm[:], eps_col, AF.Sqrt, bias=eps_col, scale=1.0)

            # per-domain stat partials + coefficient tiles (separate tiles so
            # Tile's dependency tracking stays per-domain -> early domains
            # finalize and stream output while later input is still arriving)
            p1 = [st_p.tile([P, 2 * dom_nruns[d]], f32, tag=f"p1_{d}", name=f"p1_{d}") for d in range(D)]
            p2 = [st_p.tile([P, dom_nruns[d]], f32, tag=f"p2_{d}", name=f"p2_{d}") for d in range(D)]
            NG = D // GROUP
            s1g = [st_p.tile([P, GROUP], f32, tag=f"s1g_{g}", name=f"s1g_{g}") for g in range(NG)]
            s2g = [st_p.tile([P, GROUP], f32, tag=f"s2g_{g}", name=f"s2g_{g}") for g in range(NG)]
            av = [fin_p.tile([P, GROUP], f32, tag=f"av_{g}", name=f"av_{g}") for g in range(NG)]
            bv = [fin_p.tile([P, GROUP], f32, tag=f"bv_{g}", name=f"bv_{g}") for g in range(NG)]

            def reduce_domain(d):
                # fold domain d's partials into its group-batch column
                g, j = d // GROUP, d % GROUP
                nc.vector.tensor_reduce(
                    out=s1g[g][:, j : j + 1], in_=p1[d][:], axis=X, op=A.add
                )
                nc.vector.tensor_reduce(
                    out=s2g[g][:, j : j + 1], in_=p2[d][:], axis=X, op=A.add
                )

            def finalize_group(g):
                # vectorized [128, GROUP] stats math; per-domain constants come
                # from cmat columns (built on host)
                c0 = g * GROUP
                sl = slice(c0, c0 + GROUP)
                sl8 = slice(8 + c0, 8 + c0 + GROUP)
                mneg = fin_p.tile([P, GROUP], f32, tag=f"mneg_{g}")
                nc.vector.tensor_mul(mneg[:], s1g[g][:], cm[:, sl])  # * -1/count
                ex2 = fin_p.tile([P, GROUP], f32, tag=f"ex2_{g}")
                nc.vector.tensor_mul(ex2[:], s2g[g][:], cm[:, sl8])  # * 1/count
                m2 = fin_p.tile([P, GROUP], f32, tag=f"m2_{g}")
                nc.vector.tensor_mul(m2[:], mneg[:], mneg[:])
                var = fin_p.tile([P, GROUP], f32, tag=f"var_{g}")
                nc.vector.tensor_sub(var[:], ex2[:], m2[:])
                std = fin_p.tile([P, GROUP], f32, tag=f"std_{g}")
                nc.scalar.activation(std[:], var[:], AF.Sqrt, bias=eps_col, scale=1.0)
                rstd = fin_p.tile([P, GROUP], f32, tag=f"rstd_{g}")
                nc.vector.reciprocal(rstd[:], std[:])
                a0 = fin_p.tile([P, GROUP], f32, tag=f"a0_{g}")
                nc.vector.tensor_scalar(a0[:], rstd[:], gam_col, None, A.mult)
                t1 = fin_p.tile([P, GROUP], f32, tag=f"t1_{g}")
                nc.vector.tensor_mul(t1[:], mneg[:], a0[:])
                b0 = fin_p.tile([P, GROUP], f32, tag=f"b0_{g}")
                nc.vector.tensor_scalar(b0[:], t1[:], bet_col, None, A.add)
                if all(plan.counts[c0 + j] > 1 for j in range(GROUP)):
                    nc.vector.tensor_copy(av[g][:], a0[:])
                    nc.vector.tensor_copy(bv[g][:], b0[:])
                else:  # count<=1 -> identity (out = x); count==0 unreachable
                    slm = slice(16 + c0, 16 + c0 + GROUP)
                    slm1 = slice(24 + c0, 24 + c0 + GROUP)
                    nc.vector.tensor_mul(av[g][:], a0[:], cm[:, slm])
                    nc.vector.tensor_add(av[g][:], av[g][:], cm[:, slm1])
                    nc.vector.tensor_mul(bv[g][:], b0[:], cm[:, slm])

            def pass2(ci):
                cs, cl = plan.chunks[ci]
                if resident:
                    t = xr[ci]
                else:
                    t = xin_p.tile([P, cl], fdt, tag="xin")
                    nc.sync.dma_start(t[:], xt[:, cs : cs + cl])
                ot = out_p.tile([P, cl], fdt, tag="ot")
                for rs, rl, d in chunk_runs[ci]:
                    lo = rs - cs
                    g, j = d // GROUP, d % GROUP
                    # pass2 on GpSimd: DVE and ScalarE are saturated by the
                    # stats passes; GpSimd runs the affine at ~line rate
                    nc.gpsimd.tensor_scalar(
                        out=ot[:, lo : lo + rl],
                        in0=t[:, lo : lo + rl],
                        scalar1=av[g][:, j : j + 1],
                        scalar2=bv[g][:, j : j + 1],
                        op0=A.mult,
                        op1=A.add,
                    )
                nc.sync.dma_start(outd[:, cs : cs + cl], ot[:])

            xr = {}
            ri = 0
            max_fin = -1
            next_p2 = 0
            for ci in range(nch):
                cs, cl = plan.chunks[ci]
                t = xin_p.tile([P, cl], fdt, tag=(f"xr{ci}" if resident else "xin"))
                nc.sync.dma_start(t[:], xt[:, cs : cs + cl])
                xr[ci] = t
                for rs, rl, d in chunk_runs[ci]:
                    lo = rs - cs
                    slot = run_slot[ri]
                    ri += 1
                    # split sum(x): first k cols on VectorE reduce, rest on
                    # ScalarE Copy+accum; sum(x^2) whole run on ScalarE Square
                    k = int(round(U_SPLIT * rl / ALIGN)) * ALIGN
                    if rl - k < 64:
                        k = rl
                    elif k < 64:
                        k = 0
                    if k > 0:
                        nc.vector.tensor_reduce(
                            out=p1[d][:, 2 * slot : 2 * slot + 1],
                            in_=t[:, lo : lo + k],
                            axis=X,
                            op=A.add,
                        )
                    else:
                        nc.vector.memset(p1[d][:, 2 * slot : 2 * slot + 1], 0.0)
                    if k < rl:
                        scr1 = scr_p.tile([P, CHUNK], fdt, tag="scr1")
                        nc.scalar.activation(
                            scr1[:, : rl - k],
                            t[:, lo + k : lo + rl],
                            AF.Copy,
                            accum_out=p1[d][:, 2 * slot + 1 : 2 * slot + 2],
                        )
                    else:
                        nc.vector.memset(p1[d][:, 2 * slot + 1 : 2 * slot + 2], 0.0)
                    scr2 = scr_p.tile([P, CHUNK], fdt, tag="scr2")
                    nc.scalar.activation(
                        scr2[:, :rl],
                        t[:, lo : lo + rl],
                        AF.Square,
                        accum_out=p2[d][:, slot : slot + 1],
                    )
                # fold partials for any domain whose data is now fully in;
                # run the batched stats math when a group's last domain closes
                for d in range(D):
                    if dom_last_chunk[d] == ci:
                        reduce_domain(d)
                        if d % GROUP == GROUP - 1:
                            finalize_group(d // GROUP)
                            max_fin = d
                # emit pass2 for chunks whose domains are all finalized
                while next_p2 < nch and chunk_runs[next_p2][-1][2] <= max_fin:
                    pass2(next_p2)
                    next_p2 += 1
            assert next_p2 == nch and ri == len(plan.runs)

    nc.compile()
    return nc


def _prepare(x, y, gamma, beta, mode=None):
    mode = mode or MODE
    x = np.asarray(x)
    if x.dtype != np.float32:
        x = x.astype(np.float32)
    yv = np.asarray(y)
    g = np.asarray(gamma, dtype=np.float32).reshape(-1)
    b = np.asarray(beta, dtype=np.float32).reshape(-1)
    n, f = x.shape
    assert f == P * NCORES, f"expected {P * NCORES} features, got {f}"

    key = (mode, n, f, hash(yv.tobytes()))
    if key in _cache:
        nc, plan = _cache[key]
    else:
        plan = _plan(yv)
        nc = _build(plan, mode)
        _cache.clear()
        _cache[key] = (nc, plan)

    fdtn = np.float16 if mode == "fp16" else np.float32
    # padded, domain-sorted cell matrix [npad, f]
    Xp = np.zeros((plan.npad, f), dtype=np.float32)
    Xp[plan.col_idx] = x[plan.order]
    sc = np.maximum(plan.counts, 1).astype(np.float64)
    m = (plan.counts > 1).astype(np.float32)
    cmat_base = np.concatenate(
        [(-1.0 / sc), (1.0 / sc), m, (1.0 - m)]
    ).astype(np.float32)  # [32]

    in_maps = []
    for c in range(NCORES):
        sl = slice(c * P, (c + 1) * P)
        xc = Xp[:, sl].T.astype(fdtn)  # C-contiguous [128, npad]
        cmat = np.empty((P, 35), dtype=np.float32)
        cmat[:, :32] = cmat_base[None, :]
        cmat[:, 32] = g[sl]
        cmat[:, 33] = b[sl]
        cmat[:, 34] = EPS
        in_maps.append({"xt": xc, "cmat": cmat})
    return nc, plan, in_maps, n, f


def _finish(results, plan, n, f):
    out = np.empty((n, f), dtype=np.float32)
    for c in range(NCORES):
        oc = results[c]["out"]  # [128, npad]
        out[plan.order, c * P : (c + 1) * P] = oc[:, plan.col_idx].T.astype(np.float32)
    return out


def kernel(x, y, gamma, beta):
    nc, plan, in_maps, n, f = _prepare(x, y, gamma, beta)
    res = run_bass_kernel_spmd(nc, in_maps, list(range(NCORES)))
    return _finish(res.results, plan, n, f)


def run_profiled(x, y, gamma, beta, mode=None):
    """Like kernel() but with NTFF tracing; returns (out, BassKernelResults)."""
    nc, plan, in_maps, n, f = _prepare(x, y, gamma, beta, mode=mode)
    res = run_bass_kernel_spmd(nc, in_maps, list(range(NCORES)), trace=True)
    return _finish(res.results, plan, n, f), res


# revision 11
# speedup vs baseline: 1.1222x; 1.1222x over previous
"""Domain-specific batchnorm (DSBatchNorm2 2D path) on 8 Trainium2 cores.

Strategy: feature-parallel sharding. Core c owns features [c*128,(c+1)*128).
Each core sees ALL cells for its features, so per-domain mean/var need no
cross-core reduction (counts come from the host). The host sorts cells by
domain and ships each core a transposed shard [128 features, N cells]; on
device the per-domain sums are contiguous free-axis reductions:

  pass1: per domain-run  VectorE tensor_reduce        -> sum(x)   (U_SPLIT)
                         ScalarE Copy  + accum_out    -> sum(x)   (rest)
                         ScalarE Square + accum_out   -> sum(x^2)
  per-domain finalize (emitted as soon as that domain's columns land, so
  early domains' outputs stream while later input is still arriving):
                         a = gamma*rsqrt(var+eps), b = beta - mean*a
  pass2: per domain-run  VectorE tensor_scalar(x*a + b) -> out DMA

In "fp16" mode x is shipped fp16 and stays resident in SBUF (one HBM read,
one write). In "fp32" mode everything is fp32 and pass2 re-reads from HBM.
Measured on 8 axon trn2 cores: ~124 us, rel err (absmax) ~7e-4.
"""

import os
from contextlib import ExitStack

import numpy as np

import concourse.bass as bass
import concourse.tile as tile
from concourse import bacc, mybir
from concourse.bass_utils import run_bass_kernel_spmd

N_DOMAIN = 8
EPS = 1e-5
NCORES = 8
P = 128  # SBUF partitions = features per core
CHUNK = 4096  # DMA/staging chunk (columns)
ALIGN = 8  # domain block alignment (columns)
TOTAL_ALIGN = 512
U_SPLIT = 0.78  # fraction of sum(x) columns reduced on VectorE (rest ScalarE)

MODE = os.environ.get("DSBN_MODE", "fp16")  # "fp16" | "fp32"

_cache: dict = {}


class _Plan:
    pass


def _plan(y: np.ndarray) -> _Plan:
    p = _Plan()
    y = np.asarray(y).astype(np.int64).ravel()
    n = y.shape[0]
    p.n = n
    p.counts = np.bincount(y, minlength=N_DOMAIN).astype(np.int64)
    p.order = np.argsort(y, kind="stable")
    blk = np.maximum((p.counts + ALIGN - 1) // ALIGN * ALIGN, ALIGN)
    np1 = int(blk.sum())
    npad = (np1 + TOTAL_ALIGN - 1) // TOTAL_ALIGN * TOTAL_ALIGN
    blk[-1] += npad - np1  # fold tail pad into the last domain's block
    p.npad = npad
    bstart = np.concatenate([[0], np.cumsum(blk)])[:-1]
    cstart = np.concatenate([[0], np.cumsum(p.counts)])[:-1]
    # column (padded position) of each domain-sorted row
    col_idx = np.empty(n, dtype=np.int64)
    for d in range(N_DOMAIN):
        col_idx[cstart[d] : cstart[d] + p.counts[d]] = bstart[d] + np.arange(
            p.counts[d]
        )
    p.col_idx = col_idx
    # chunks
    chunks = []
    cs = 0
    while cs < npad:
        cl = min(CHUNK, npad - cs)
        chunks.append((cs, cl))
        cs += cl
    p.chunks = chunks
    # runs = intersections of domain blocks with chunks, in column order
    runs = []  # (col_start, col_len, domain, chunk_index)
    dom_runs = [[] for _ in range(N_DOMAIN)]
    for ci, (cs, cl) in enumerate(chunks):
        ce = cs + cl
        for d in range(N_DOMAIN):
            rs = max(cs, int(bstart[d]))
            re_ = min(ce, int(bstart[d] + blk[d]))
            if rs < re_:
                dom_runs[d].append(len(runs))
                runs.append((rs, re_ - rs, d, ci))
    for d in range(N_DOMAIN):
        rr = dom_runs[d]
        assert rr == list(range(rr[0], rr[-1] + 1))
    p.runs = runs
    p.dom_runs = [(rr[0], rr[-1] + 1) for rr in dom_runs]
    return p


def _build(plan: _Plan, mode: str):
    fdt = mybir.dt.float16 if mode == "fp16" else mybir.dt.float32
    f32 = mybir.dt.float32
    A = mybir.AluOpType
    AF = mybir.ActivationFunctionType
    X = mybir.AxisListType.X
    resident = mode == "fp16"
    npad = plan.npad
    D = N_DOMAIN

    # per-chunk run lists and per-domain bookkeeping
    nch = len(plan.chunks)
    chunk_runs = [[] for _ in range(nch)]
    dom_nruns = [0] * D
    run_slot = []  # index of this run within its domain
    for rs, rl, d, ci in plan.runs:
        chunk_runs[ci].append((rs, rl, d))
        run_slot.append(dom_nruns[d])
        dom_nruns[d] += 1
    dom_last_chunk = [max(ci for rs, rl, dd, ci in plan.runs if dd == d) for d in range(D)]

    nc = bacc.Bacc("TRN2", target_bir_lowering=False, debug=False, num_devices=NCORES)
    xt = nc.dram_tensor("xt", [P, npad], fdt, kind="ExternalInput").ap()
    cmat = nc.dram_tensor("cmat", [P, 35], f32, kind="ExternalInput").ap()
    outd = nc.dram_tensor("out", [P, npad], fdt, kind="ExternalOutput").ap()

    with tile.TileContext(nc) as tc:
        with ExitStack() as ctx:
            const_p = ctx.enter_context(tc.tile_pool(name="const", bufs=1))
            xin_p = ctx.enter_context(
                tc.tile_pool(name="xin", bufs=1 if resident else 3)
            )
            scr_p = ctx.enter_context(tc.tile_pool(name="scr", bufs=2))
            st_p = ctx.enter_context(tc.tile_pool(name="st", bufs=1))
            fin_p = ctx.enter_context(tc.tile_pool(name="fin", bufs=1))
            out_p = ctx.enter_context(tc.tile_pool(name="ot", bufs=3))

            cm = const_p.tile([P, 35], f32, tag="cm")
            nc.sync.dma_start(cm[:], cmat)
            gam_col = cm[:, 32:33]
            bet_col = cm[:, 33:34]
            eps_col = cm[:, 34:35]

            # dummy Sqrt up front: pulls the ACT table load into the DMA ramp
            warm = const_p.tile([P, 1], f32, tag="warm")
            nc.scalar.activation(warm[:], eps_col, AF.Sqrt, bias=eps_col, scale=1.0)

            # per-domain stat partials + coefficient tiles (separate tiles so
            # Tile's dependency tracking stays per-domain -> early domains
            # finalize and stream output while later input is still arriving)
            p1 = [st_p.tile([P, 2 * dom_nruns[d]], f32, tag=f"p1_{d}", name=f"p1_{d}") for d in range(D)]
            p2 = [st_p.tile([P, dom_nruns[d]], f32, tag=f"p2_{d}", name=f"p2_{d}") for d in range(D)]
            av = [fin_p.tile([P, 1], f32, tag=f"av_{d}", name=f"av_{d}") for d in range(D)]
            bv = [fin_p.tile([P, 1], f32, tag=f"bv_{d}", name=f"bv_{d}") for d in range(D)]

            def finalize(d):
                c = float(plan.counts[d])
                if c <= 1.0:
                    nc.vector.memset(av[d][:], 1.0)
                    nc.vector.memset(bv[d][:], 0.0)
                    return
                s1 = fin_p.tile([P, 1], f32, tag=f"s1_{d}")
                nc.vector.tensor_reduce(out=s1[:], in_=p1[d][:], axis=X, op=A.add)
                s2 = fin_p.tile([P, 1], f32, tag=f"s2_{d}")
                nc.vector.tensor_reduce(out=s2[:], in_=p2[d][:], axis=X, op=A.add)
                mneg = fin_p.tile([P, 1], f32, tag=f"mneg_{d}")
                nc.vector.tensor_scalar(mneg[:], s1[:], -1.0 / c, None, A.mult)
                ex2 = fin_p.tile([P, 1], f32, tag=f"ex2_{d}")
                nc.vector.tensor_scalar(ex2[:], s2[:], 1.0 / c, None, A.mult)
                m2 = fin_p.tile([P, 1], f32, tag=f"m2_{d}")
                nc.vector.tensor_mul(m2[:], mneg[:], mneg[:])
                var = fin_p.tile([P, 1], f32, tag=f"var_{d}")
                nc.vector.tensor_sub(var[:], ex2[:], m2[:])
                std = fin_p.tile([P, 1], f32, tag=f"std_{d}")
                nc.scalar.activation(std[:], var[:], AF.Sqrt, bias=eps_col, scale=1.0)
                rstd = fin_p.tile([P, 1], f32, tag=f"rstd_{d}")
                nc.vector.reciprocal(rstd[:], std[:])
                nc.vector.tensor_scalar(av[d][:], rstd[:], gam_col, None, A.mult)
                t1 = fin_p.tile([P, 1], f32, tag=f"t1_{d}")
                nc.vector.tensor_mul(t1[:], mneg[:], av[d][:])
                nc.vector.tensor_scalar(bv[d][:], t1[:], bet_col, None, A.add)

            def pass2(ci):
                cs, cl = plan.chunks[ci]
                if resident:
                    t = xr[ci]
                else:
                    t = xin_p.tile([P, cl], fdt, tag="xin")
                    nc.sync.dma_start(t[:], xt[:, cs : cs + cl])
                ot = out_p.tile([P, cl], fdt, tag="ot")
                for rs, rl, d in chunk_runs[ci]:
                    lo = rs - cs
                    nc.vector.tensor_scalar(
                        out=ot[:, lo : lo + rl],
                        in0=t[:, lo : lo + rl],
                        scalar1=av[d][:, 0:1],
                        scalar2=bv[d][:, 0:1],
                        op0=A.mult,
                        op1=A.add,
                    )
                nc.sync.dma_start(outd[:, cs : cs + cl], ot[:])

            xr = {}
            ri = 0
            max_fin = -1
            next_p2 = 0
            for ci in range(nch):
                cs, cl = plan.chunks[ci]
                t = xin_p.tile([P, cl], fdt, tag=(f"xr{ci}" if resident else "xin"))
                nc.sync.dma_start(t[:], xt[:, cs : cs + cl])
                xr[ci] = t
                for rs, rl, d in chunk_runs[ci]:
                    lo = rs - cs
                    slot = run_slot[ri]
                    ri += 1
                    # split sum(x): first k cols on VectorE reduce, rest on
                    # ScalarE Copy+accum; sum(x^2) whole run on ScalarE Square
                    k = int(round(U_SPLIT * rl / ALIGN)) * ALIGN
                    if rl - k < 64:
                        k = rl
                    elif k < 64:
                        k = 0
                    if k > 0:
                        nc.vector.tensor_reduce(
                            out=p1[d][:, 2 * slot : 2 * slot + 1],
                            in_=t[:, lo : lo + k],
                            axis=X,
                            op=A.add,
                        )
                    else:
                        nc.vector.memset(p1[d][:, 2 * slot : 2 * slot + 1], 0.0)
                    if k < rl:
                        scr1 = scr_p.tile([P, CHUNK], fdt, tag="scr1")
                        nc.scalar.activation(
                            scr1[:, : rl - k],
                            t[:, lo + k : lo + rl],
                            AF.Copy,
                            accum_out=p1[d][:, 2 * slot + 1 : 2 * slot + 2],
                        )
                    else:
                        nc.vector.memset(p1[d][:, 2 * slot + 1 : 2 * slot + 2], 0.0)
                    scr2 = scr_p.tile([P, CHUNK], fdt, tag="scr2")
                    nc.scalar.activation(
                        scr2[:, :rl],
                        t[:, lo : lo + rl],
                        AF.Square,
                        accum_out=p2[d][:, slot : slot + 1],
                    )
                # finalize any domain whose data is now fully in
                for d in range(D):
                    if dom_last_chunk[d] == ci:
                        finalize(d)
                        max_fin = d
                # emit pass2 for chunks whose domains are all finalized
                while next_p2 < nch and chunk_runs[next_p2][-1][2] <= max_fin:
                    pass2(next_p2)
                    next_p2 += 1
            assert next_p2 == nch and ri == len(plan.runs)

    nc.compile()
    return nc


def _prepare(x, y, gamma, beta, mode=None):
    mode = mode or MODE
    x = np.asarray(x)
    if x.dtype != np.float32:
        x = x.astype(np.float32)
    yv = np.asarray(y)
    g = np.asarray(gamma, dtype=np.float32).reshape(-1)
    b = np.asarray(beta, dtype=np.float32).reshape(-1)
    n, f = x.shape
    assert f == P * NCORES, f"expected {P * NCORES} features, got {f}"

    key = (mode, n, f, hash(yv.tobytes()))
    if key in _cache:
        nc, plan = _cache[key]
    else:
        plan = _plan(yv)
        nc = _build(plan, mode)
        _cache.clear()
        _cache[key] = (nc, plan)

    fdtn = np.float16 if mode == "fp16" else np.float32
    # padded, domain-sorted cell matrix [npad, f]
    Xp = np.zeros((plan.npad, f), dtype=np.float32)
    Xp[plan.col_idx] = x[plan.order]
    sc = np.maximum(plan.counts, 1).astype(np.float64)
    m = (plan.counts > 1).astype(np.float32)
    cmat_base = np.concatenate(
        [(-1.0 / sc), (1.0 / sc), m, (1.0 - m)]
    ).astype(np.float32)  # [32]

    in_maps = []
    for c in range(NCORES):
        sl = slice(c * P, (c + 1) * P)
        xc = Xp[:, sl].T.astype(fdtn)  # C-contiguous [128, npad]
        cmat = np.empty((P, 35), dtype=np.float32)
        cmat[:, :32] = cmat_base[None, :]
        cmat[:, 32] = g[sl]
        cmat[:, 33] = b[sl]
        cmat[:, 34] = EPS
        in_maps.append({"xt": xc, "cmat": cmat})
    return nc, plan, in_maps, n, f


def _finish(results, plan, n, f):
    out = np.empty((n, f), dtype=np.float32)
    for c in range(NCORES):
        oc = results[c]["out"]  # [128, npad]
        out[plan.order, c * P : (c + 1) * P] = oc[:, plan.col_idx].T.astype(np.float32)
    return out


def kernel(x, y, gamma, beta):
    nc, plan, in_maps, n, f = _prepare(x, y, gamma, beta)
    res = run_bass_kernel_spmd(nc, in_maps, list(range(NCORES)))
    return _finish(res.results, plan, n, f)


def run_profiled(x, y, gamma, beta, mode=None):
    """Like kernel() but with NTFF tracing; returns (out, BassKernelResults)."""
    nc, plan, in_maps, n, f = _prepare(x, y, gamma, beta, mode=mode)
    res = run_bass_kernel_spmd(nc, in_maps, list(range(NCORES)), trace=True)
    return _finish(res.results, plan, n, f), res
